# revision 4
# baseline (speedup 1.0000x reference)
"""Trainium2 Bass kernel for GAT-style GNN message passing (8 NeuronCores).

Math (matches reference):
    z = concat([m_sim @ Wm.T, d_sim @ Wd.T])           # [N, F]
    e = leaky_relu(sum(z[src] * z[dst], -1), 0.2)      # [E]
    alpha = softmax of e over incoming edges of dst
    h = elu(segment_sum(alpha[:,None] * z[src], dst))  # [N, F]

Softmax max-subtraction is replaced by a clamp at 80 (only self-loops exceed
80 and the clamp acts as a per-segment shift).

Layout: nodes range-partitioned over 8 cores (12500 each, padded to 12544).
Per core, edges are grouped into sub-rows (dst node x src window); src windows
are 32768 padded-z rows (int16 index range), so 3 full windows + one 2048-row
tail window cover all 100352 rows -> fewer sub-row splits than 4x25088.
Per window, sub-rows are sorted by length and packed 128-per-tile with the
tile width w = longest sub-row in the tile; tile/width structure is unified
across cores (max) so one compiled module serves all 8 cores SPMD.

Edge phase: one dma_gather per window preloads all zdst rows (queue 1); per
(w,B)-batch one dma_gather fetches zsrc (queues 0/3 alternating, up to 7168
idx/call); e = reduce_f(zsrc*zdst_bcast) in fp32, leaky+clamp, exp (Act);
payload converted bf16 (Act), weighted by ex and tree-added (DVE 2x);
partials [128, 65] bf16 staged 16 tiles at a time and combined across windows
with SBUF-destination dma_scatter_add (parity-split CCE add, bf16 payload =
half descriptor cost; idx = local node id; partition n%128, buf parity
(n>>7)&1, column n>>8).

Padding slots point at all-zero z rows so exp(0)=1; the denominator is
corrected (in fp32, before the bf16 round) by the host-provided pad count.
"""

import numpy as np
from contextlib import ExitStack

import concourse.bass as bass
import concourse.tile as tile
from concourse import bacc, mybir
from concourse import bass_utils

# ---- problem constants (hardcoded; kernel.py must be self-contained) ----
N = 100000
F = 64
E = 1600000
C = 8                 # cores
NPC = N // C          # nodes per core = 12500
R = 12544             # padded rows per core block (junk rows 12500..12543)
RT = C * R            # total padded z rows = 100352
WB = [0, 32768, 65536, 98304, RT]   # src window bounds (rows)
NW = 4                # number of src windows
# one all-zero row inside each window (core-junk rows), window-local:
JROW_WIN = [12500, 2 * R + 12500 - WB[1], 5 * R + 12500 - WB[2],
            7 * R + 12500 - WB[3]]
SLOPE = 0.2
DM = 256              # feature dim of m_sim / d_sim
GSLOT = 49            # accumulator free-dim groups per parity buffer
WBMAX = 56            # max zsrc columns per compute batch
BTMAX = 8             # max tiles per compute batch
SGMAX = 32            # tiles per scatter group (4096-idx scatter calls)
GCALL = 1024          # max gather indices per call (hard ucode limit)

_nc_cache = {}


def _wrap16(idx16):
    """[n] int16 -> [128, n/16]: token i at [i%16, i//16], replicated to the
    8 Q7-core partition groups."""
    n = idx16.shape[0]
    w = np.ascontiguousarray(idx16.reshape(n // 16, 16).T)
    return np.tile(w, (8, 1))


# --------------------------------------------------------------------------
# host-side index preparation
# --------------------------------------------------------------------------

def _prep(src, dst):
    """Build the shared tile structure and per-core index streams.

    Returns (wseq, src16, q16, padf):
      wseq: per window, list of tile widths (shared across cores)
      src16[c]: [128, T/16] int16 wrapped gather idx per token
      q16[c]:   [128, Qp/16] int16 wrapped node idx per sub-row
      padf[c]:  [128, Qp/128] float32 pad count per sub-row
    """
    src = np.asarray(src).astype(np.int64)
    dst = np.asarray(dst).astype(np.int64)
    core = dst // NPC
    dloc = dst - core * NPC
    srcpad = (src // NPC) * R + (src % NPC)
    win = np.searchsorted(np.asarray(WB[1:4]), srcpad, side="right")
    sloc = (srcpad - np.asarray(WB)[win]).astype(np.int16)

    key = (core * NW + win) * NPC + dloc
    order = np.argsort(key, kind="stable")
    ks = key[order]
    sloc_s = sloc[order]
    # rank of each edge within its (core, win, node) run
    first = np.r_[True, ks[1:] != ks[:-1]]
    grp_start = np.maximum.accumulate(np.where(first, np.arange(E), 0))
    rank = np.arange(E) - grp_start

    uniq, counts = np.unique(ks, return_counts=True)
    c_u = uniq // (NW * NPC)
    k_u = (uniq // NPC) % NW
    n_u = uniq % NPC

    # per (core, win): sorted sub-row lengths
    lens = {}
    nodes = {}
    for c in range(C):
        for k in range(NW):
            m = (c_u == c) & (k_u == k)
            cnt = counts[m]
            nds = n_u[m]
            o = np.argsort(-cnt, kind="stable")
            lens[(c, k)] = cnt[o]
            nodes[(c, k)] = nds[o]

    # global tile structure
    wseq = []
    for k in range(NW):
        ntile = max(-(-len(lens[(c, k)]) // 128) for c in range(C))
        ws = []
        for g in range(ntile):
            w = 1
            for c in range(C):
                L = lens[(c, k)]
                if g * 128 < len(L):
                    w = max(w, int(L[g * 128]))
            ws.append(w)
        wseq.append(ws)

    tokbase = [0]
    qbase = [0]
    for k in range(NW):
        tokbase.append(tokbase[-1] + 128 * sum(wseq[k]))
        qbase.append(qbase[-1] + 128 * len(wseq[k]))
    T, Qp = tokbase[-1], qbase[-1]

    src16 = []
    q16 = []
    padf = []
    for c in range(C):
        s16 = np.empty(T, dtype=np.int16)
        for k in range(NW):
            s16[tokbase[k]:tokbase[k + 1]] = JROW_WIN[k]
        qq = np.empty(Qp, dtype=np.int16)
        qar = np.arange(Qp)
        qq[:] = (12500 + qar % 44).astype(np.int16)
        pf = np.zeros(Qp, dtype=np.float32)
        for k in range(NW):
            L = lens[(c, k)]
            Nd = nodes[(c, k)]
            nsub = len(L)
            ws = np.asarray(wseq[k], dtype=np.int64)
            cumw = np.concatenate([[0], np.cumsum(ws)])
            # per sub-row q: tile g = q//128, partition p = q%128
            q_ids = np.arange(nsub)
            g_ids = q_ids // 128
            qq[qbase[k] + q_ids] = Nd.astype(np.int16)
            pf[qbase[k] + q_ids] = (ws[g_ids] - L).astype(np.float32)
            pf[qbase[k] + nsub:qbase[k + 1]] = 0.0  # junk rows: denom junk anyway
            # per edge: token = tokbase[k] + (cumw[g] + rank)*128 + p
            lo = np.searchsorted(ks, (c * NW + k) * NPC)
            hi = np.searchsorted(ks, (c * NW + k + 1) * NPC)
            if hi > lo:
                nd_e = ks[lo:hi] % NPC            # node per edge
                r_e = rank[lo:hi]
                inv = np.empty(NPC, dtype=np.int64)
                inv[Nd] = q_ids
                q_e = inv[nd_e]
                tok = (tokbase[k]
                       + (cumw[q_e // 128] + r_e) * 128 + (q_e % 128))
                s16[tok] = sloc_s[lo:hi]
        src16.append(_wrap16(s16))
        q16.append(_wrap16(qq))
        padf.append(np.ascontiguousarray(
            pf.reshape(Qp // 128, 128).T).astype(np.float32))
    return wseq, src16, q16, padf


def _batches(wseq_k):
    """Split a window's tile-width run-lengths into (w, B) compute batches."""
    out = []
    i = 0
    while i < len(wseq_k):
        w = wseq_k[i]
        run = 1
        while i + run < len(wseq_k) and wseq_k[i + run] == w:
            run += 1
        maxb = max(1, min(BTMAX, WBMAX // w))
        for b0 in range(0, run, maxb):
            out.append((w, min(maxb, run - b0)))
        i += run
    return out


# --------------------------------------------------------------------------
# launch 1: projection  z_c [R, F] = xT_c.T @ wT_c  (pipelined x loads)
# --------------------------------------------------------------------------

def _build_proj_nc():
    nc = bacc.Bacc("TRN2", target_bir_lowering=False, debug=False,
                   num_devices=C)
    xT = nc.dram_tensor("xT", [DM, R], mybir.dt.float32,
                        kind="ExternalInput").ap()
    wT = nc.dram_tensor("wT", [DM, F], mybir.dt.float32,
                        kind="ExternalInput").ap()
    z_out = nc.dram_tensor("z", [R, F], mybir.dt.float32,
                           kind="ExternalOutput").ap()

    with tile.TileContext(nc) as tc:
        with ExitStack() as ctx:
            wp = ctx.enter_context(tc.tile_pool(name="w", bufs=1))
            xp = ctx.enter_context(tc.tile_pool(name="x", bufs=3))
            pp = ctx.enter_context(tc.tile_pool(name="ps", bufs=8,
                                                space="PSUM"))
            op = ctx.enter_context(tc.tile_pool(name="o", bufs=2))

            wt = wp.tile([128, 2, F], mybir.dt.float32)
            for j in range(2):
                nc.sync.dma_start(wt[:, j, :], wT[j * 128:(j + 1) * 128, :])

            ntiles = R // 128
            SB = 8
            z_r = z_out.rearrange("(t p) f -> p t f", p=128)
            for r0 in range(0, ntiles, SB):
                sb = min(SB, ntiles - r0)
                xt = xp.tile([128, 2, SB * 128], mybir.dt.float32, tag="xt")
                for j in range(2):
                    nc.sync.dma_start(
                        xt[:, j, :sb * 128],
                        xT[j * 128:(j + 1) * 128,
                           r0 * 128:(r0 + sb) * 128])
                ot = op.tile([128, SB, F], mybir.dt.float32, tag="ot")
                for t in range(sb):
                    ps = pp.tile([128, F], mybir.dt.float32, tag="ps")
                    for j in range(2):
                        nc.tensor.matmul(
                            out=ps[:],
                            lhsT=xt[:, j, t * 128:(t + 1) * 128],
                            rhs=wt[:, j, :],
                            start=(j == 0), stop=(j == 1))
                    nc.vector.tensor_copy(ot[:, t, :], ps[:])
                nc.sync.dma_start(z_r[:, r0:r0 + sb, :], ot[:, :sb, :])
    nc.compile()
    return nc


# --------------------------------------------------------------------------
# launch 2: edge phase
# --------------------------------------------------------------------------

def _build_edge_nc(wseq):
    T = 128 * sum(sum(ws) for ws in wseq)
    Qp = 128 * sum(len(ws) for ws in wseq)
    QTMAX = max(len(ws) for ws in wseq)

    nc = bacc.Bacc("TRN2", target_bir_lowering=False, debug=False,
                   num_devices=C, num_swdge_queues=4)
    z8 = nc.dram_tensor("z8", [RT, F], mybir.dt.float32,
                        kind="ExternalInput").ap()
    zown = nc.dram_tensor("zown", [R, F], mybir.dt.float32,
                          kind="ExternalInput").ap()
    s16d = nc.dram_tensor("s16", [128, T // 16], mybir.dt.int16,
                          kind="ExternalInput").ap()
    q16d = nc.dram_tensor("q16", [128, Qp // 16], mybir.dt.int16,
                          kind="ExternalInput").ap()
    padd = nc.dram_tensor("pad", [128, Qp // 128], mybir.dt.float32,
                          kind="ExternalInput").ap()
    h2 = nc.dram_tensor("h2", [2, 128, GSLOT, F], mybir.dt.float32,
                        kind="ExternalOutput").ap()

    qbase = [0]
    for k in range(NW):
        qbase.append(qbase[-1] + len(wseq[k]))

    with tile.TileContext(nc) as tc:
        with ExitStack() as ctx:
            ip = ctx.enter_context(tc.tile_pool(name="idx", bufs=1))
            ap_ = ctx.enter_context(tc.tile_pool(name="acc", bufs=1))
            dp = ctx.enter_context(tc.tile_pool(name="d", bufs=1))
            gp = ctx.enter_context(tc.tile_pool(name="g", bufs=3))
            ep = ctx.enter_context(tc.tile_pool(name="e", bufs=2))
            stp = ctx.enter_context(tc.tile_pool(name="st", bufs=3))

            # idx preload, per-window chunks so window 0 starts early
            s16t = ip.tile([128, T // 16], mybir.dt.int16, name="s16t")
            q16t = ip.tile([128, Qp // 16], mybir.dt.int16, name="q16t")
            padt = ip.tile([128, Qp // 128], mybir.dt.float32, name="padt")
            tb = 0
            qb = 0
            for k in range(NW):
                tk = 128 * sum(wseq[k])
                qk = 128 * len(wseq[k])
                nc.sync.dma_start(s16t[:, tb // 16:(tb + tk) // 16],
                                  s16d[:, tb // 16:(tb + tk) // 16])
                nc.sync.dma_start(q16t[:, qb // 16:(qb + qk) // 16],
                                  q16d[:, qb // 16:(qb + qk) // 16])
                nc.sync.dma_start(padt[:, qb // 128:(qb + qk) // 128],
                                  padd[:, qb // 128:(qb + qk) // 128])
                tb += tk
                qb += qk

            # accumulators: [rep][parity], rep alternates per scatter group
            acc = [[ap_.tile([128, GSLOT, F + 1], mybir.dt.bfloat16,
                             name=f"acc{r}{p}") for p in range(2)]
                   for r in range(2)]
            for r in range(2):
                for p in range(2):
                    nc.gpsimd.memset(acc[r][p][:], 0.0)

            tok_col = 0
            sgrp = 0
            grp_q0 = 0
            grp_j = 0
            st = None

            def flush_scatter():
                nonlocal sgrp, grp_j, grp_q0, st
                if sgrp == 0:
                    return
                nc.gpsimd.dma_scatter_add(
                    acc[grp_j % 2][0][:], st[:, :sgrp, :],
                    q16t[:, grp_q0 * 8:(grp_q0 + sgrp) * 8],
                    sgrp * 128, sgrp * 128, F + 1,
                    queue_num=2, sbuf_tokens_per_rank=128,
                    parity_reg=0, out_ap_other=acc[grp_j % 2][1][:])
                grp_j += 1
                sgrp = 0
                st = None

            gci = 0   # src gather call counter (queue 0/3 alternation)
            for k in range(NW):
                qt_k = len(wseq[k])
                # preload all zdst rows of this window (queue 1, 1024/call)
                wt = dp.tile([128, QTMAX, F], mybir.dt.float32, tag="wt")
                for c0 in range(0, qt_k, GCALL // 128):
                    cb = min(GCALL // 128, qt_k - c0)
                    nc.gpsimd.dma_gather(
                        wt[:, c0:c0 + cb, :], zown[:, :],
                        q16t[:, (qbase[k] + c0) * 8:(qbase[k] + c0 + cb) * 8],
                        cb * 128, cb * 128, F, queue_num=1)

                q_off = 0   # tile offset within this window
                for w, B in _batches(wseq[k]):
                    wsum = B * w
                    zsrc = gp.tile([128, WBMAX, F], mybir.dt.float32,
                                   tag="zsrc")
                    for c0 in range(0, wsum, GCALL // 128):
                        nn = min(GCALL // 128, wsum - c0) * 128
                        i0 = (tok_col + c0) * 8
                        nc.gpsimd.dma_gather(
                            zsrc[:, c0:c0 + nn // 128, :],
                            z8[WB[k]:WB[k + 1], :],
                            s16t[:, i0:i0 + nn // 16], nn, nn, F,
                            queue_num=(0, 3)[gci % 2])
                        gci += 1

                    prod = ep.tile([128, WBMAX, F], mybir.dt.float32,
                                   tag="prod")
                    zsrc4 = zsrc[:, :wsum, :].rearrange(
                        "p (b w) f -> p b w f", b=B)
                    prod4 = prod[:, :wsum, :].rearrange(
                        "p (b w) f -> p b w f", b=B)
                    zdst4 = wt[:, q_off:q_off + B, :].rearrange(
                        "p b (o f) -> p b o f", o=1)
                    nc.vector.tensor_mul(
                        prod4, zsrc4, zdst4.to_broadcast([128, B, w, F]))
                    ex = ep.tile([128, WBMAX], mybir.dt.float32, tag="ex")
                    nc.vector.tensor_reduce(
                        ex[:, :wsum], prod[:, :wsum, :],
                        axis=mybir.AxisListType.X, op=mybir.AluOpType.add)
                    nc.vector.scalar_tensor_tensor(
                        out=ex[:, :wsum], in0=ex[:, :wsum], scalar=SLOPE,
                        in1=ex[:, :wsum],
                        op0=mybir.AluOpType.mult, op1=mybir.AluOpType.max)
                    nc.vector.tensor_scalar_min(ex[:, :wsum], ex[:, :wsum],
                                                80.0)
                    nc.scalar.activation(ex[:, :wsum], ex[:, :wsum],
                                         mybir.ActivationFunctionType.Exp)
                    # bf16 payload path: convert on Act, multiply+tree on DVE
                    # at 2x; denominators read the same bf16 ex so softmax
                    # weights stay numerator/denominator-consistent.
                    zb = ep.tile([128, WBMAX, F], mybir.dt.bfloat16,
                                 tag="zb")
                    nc.scalar.copy(zb[:, :wsum, :], zsrc[:, :wsum, :])
                    exb = ep.tile([128, WBMAX], mybir.dt.bfloat16, tag="exb")
                    nc.scalar.copy(exb[:, :wsum], ex[:, :wsum])
                    with nc.allow_low_precision(reason="bf16 payload"):
                        nc.vector.tensor_mul(
                            zb[:, :wsum, :], zb[:, :wsum, :],
                            exb[:, :wsum].to_broadcast([128, wsum, F]))

                    if st is not None and sgrp + B > SGMAX:
                        flush_scatter()
                    if st is None:
                        st = stp.tile([128, SGMAX, F + 1],
                                      mybir.dt.bfloat16, tag="st")
                        grp_q0 = qbase[k] + q_off
                    zb4 = zb[:, :wsum, :].rearrange(
                        "p (b w) f -> p b w f", b=B)
                    cur_w = w
                    with nc.allow_low_precision(reason="bf16 tree"):
                        while cur_w > 1:
                            hh = cur_w // 2
                            nc.vector.tensor_add(
                                zb4[:, :, 0:hh, :],
                                zb4[:, :, 0:hh, :],
                                zb4[:, :, cur_w - hh:cur_w, :])
                            cur_w -= hh
                    nc.scalar.copy(st[:, sgrp:sgrp + B, 0:F],
                                   zb4[:, :, 0, :])
                    # denominator in fp32, pad-correct, then round to bf16
                    dn = ep.tile([128, BTMAX], mybir.dt.float32, tag="dn")
                    nc.vector.tensor_reduce(
                        dn[:, :B],
                        exb[:, :wsum].rearrange("p (b w) -> p b w", b=B),
                        axis=mybir.AxisListType.X,
                        op=mybir.AluOpType.add)
                    nc.vector.tensor_sub(
                        dn[:, :B], dn[:, :B],
                        padt[:, qbase[k] + q_off:qbase[k] + q_off + B])
                    with nc.allow_low_precision(reason="bf16 partials"):
                        nc.vector.tensor_copy(st[:, sgrp:sgrp + B, F],
                                              dn[:, :B])
                    sgrp += B
                    if sgrp == SGMAX:
                        flush_scatter()
                    q_off += B
                    tok_col += wsum
                flush_scatter()   # window boundary: avoid cross-window dups

            # ---- normalize + elu + store (reuse compute pools) ----
            for p in range(2):
                a = acc[0][p]
                with nc.allow_low_precision(reason="bf16 acc merge"):
                    nc.vector.tensor_add(a[:], a[:], acc[1][p][:])
                rec = ep.tile([128, WBMAX], mybir.dt.float32, tag="ex")
                nc.vector.tensor_copy(rec[:, :GSLOT], a[:, :, F])
                nc.vector.tensor_scalar_max(rec[:, :GSLOT], rec[:, :GSLOT],
                                            1e-30)
                nc.vector.reciprocal(rec[:, :GSLOT], rec[:, :GSLOT])
                hb = gp.tile([128, WBMAX, F], mybir.dt.float32, tag="zsrc")
                h = hb[:, :GSLOT, :]
                nc.vector.tensor_copy(h, a[:, :, 0:F])
                nc.vector.tensor_mul(
                    h, h, rec[:, :GSLOT].to_broadcast([128, GSLOT, F]))
                nb = ep.tile([128, WBMAX, F], mybir.dt.float32, tag="prod")
                hneg = nb[:, :GSLOT, :]
                nc.vector.tensor_scalar_min(hneg, h, 0.0)
                nc.scalar.activation(hneg, hneg,
                                     mybir.ActivationFunctionType.Exp)
                nc.vector.tensor_scalar_max(h, h, 0.0)
                nc.vector.tensor_add(h, h, hneg)
                nc.vector.tensor_scalar_add(h, h, -1.0)
                nc.sync.dma_start(h2[p, :, :, :], h)
    nc.compile()
    return nc


# --------------------------------------------------------------------------
# entry point
# --------------------------------------------------------------------------

def kernel(m_sim, d_sim, Wm, Wd, src, dst, _profile=None):
    m_sim = np.asarray(m_sim, dtype=np.float32)
    d_sim = np.asarray(d_sim, dtype=np.float32)
    Wm = np.asarray(Wm, dtype=np.float32)
    Wd = np.asarray(Wd, dtype=np.float32)

    wseq, src16, q16, padf = _prep(src, dst)

    # ---- launch 1: projection ----
    if "proj" not in _nc_cache:
        _nc_cache["proj"] = _build_proj_nc()
    proj_nc = _nc_cache["proj"]

    x = np.concatenate([m_sim, d_sim], axis=0)        # [N, DM]
    wmT = np.ascontiguousarray(Wm.T)                  # [DM, F]
    wdT = np.ascontiguousarray(Wd.T)
    in1 = []
    for c in range(C):
        xT_c = np.zeros((DM, R), dtype=np.float32)
        xT_c[:, :NPC] = x[c * NPC:(c + 1) * NPC].T
        in1.append({"xT": xT_c, "wT": wmT if c < 4 else wdT})
    r1 = bass_utils.run_bass_kernel_spmd(proj_nc, in1,
                                         core_ids=list(range(C)),
                                         **(_profile or {}))
    z8_full = np.concatenate([r1.results[c]["z"] for c in range(C)],
                             axis=0)                  # [RT, F]

    # ---- launch 2: edge phase ----
    key = ("edge", tuple(tuple(ws) for ws in wseq))
    if key not in _nc_cache:
        _nc_cache[key] = _build_edge_nc(wseq)
    edge_nc = _nc_cache[key]

    in2 = []
    for c in range(C):
        in2.append({
            "z8": z8_full,
            "zown": z8_full[c * R:(c + 1) * R],
            "s16": src16[c],
            "q16": q16[c],
            "pad": padf[c],
        })
    r2 = bass_utils.run_bass_kernel_spmd(edge_nc, in2,
                                         core_ids=list(range(C)),
                                         **(_profile or {}))
    h = np.empty((N, F), dtype=np.float32)
    for c in range(C):
        h2 = r2.results[c]["h2"]                      # [2, 128, GSLOT, 64]
        # node n (local) at [ (n>>7)&1, n&127, n>>8 ]
        n_ = np.arange(NPC)
        h[c * NPC:(c + 1) * NPC] = h2[(n_ >> 7) & 1, n_ & 127, n_ >> 8]
    kernel._last_results = (r1, r2)
    return h


# revision 9
# speedup vs baseline: 1.1044x; 1.1044x over previous
"""Trainium2 Bass kernel for GAT-style GNN message passing (8 NeuronCores).

Math (matches reference):
    z = concat([m_sim @ Wm.T, d_sim @ Wd.T])           # [N, F]
    e = leaky_relu(sum(z[src] * z[dst], -1), 0.2)      # [E]
    alpha = softmax of e over incoming edges of dst
    h = elu(segment_sum(alpha[:,None] * z[src], dst))  # [N, F]

Softmax max-subtraction is replaced by a clamp at 80 (only self-loops exceed
80 and the clamp acts as a per-segment shift).

Layout: nodes range-partitioned over 8 cores (12500 each, padded to 12544).
Per core, edges are grouped into sub-rows (dst node x src window); src windows
are 32768 padded-z rows (int16 index range), so 3 full windows + one 2048-row
tail window cover all 100352 rows -> fewer sub-row splits than 4x25088.
Per window, sub-rows are sorted by length and packed 128-per-tile with the
tile width w = longest sub-row in the tile; tile/width structure is unified
across cores (max) so one compiled module serves all 8 cores SPMD.

Edge phase: one dma_gather per window preloads all zdst rows (queue 1); per
(w,B)-batch one dma_gather fetches zsrc (queues 0/3 alternating, up to 7168
idx/call); e = reduce_f(zsrc*zdst_bcast) in fp32, leaky+clamp, exp (Act);
payload converted bf16 (Act), weighted by ex and tree-added (DVE 2x);
partials [128, 65] bf16 staged 16 tiles at a time and combined across windows
with SBUF-destination dma_scatter_add (parity-split CCE add, bf16 payload =
half descriptor cost; idx = local node id; partition n%128, buf parity
(n>>7)&1, column n>>8).

Padding slots point at all-zero z rows so exp(0)=1; the denominator is
corrected (in fp32, before the bf16 round) by the host-provided pad count.
"""

import numpy as np
from contextlib import ExitStack

import concourse.bass as bass
import concourse.tile as tile
from concourse import bacc, mybir
from concourse import bass_utils

# ---- problem constants (hardcoded; kernel.py must be self-contained) ----
N = 100000
F = 64
E = 1600000
C = 8                 # cores
NPC = N // C          # nodes per core = 12500
R = 12544             # padded rows per core block (junk rows 12500..12543)
RT = C * R            # total padded z rows = 100352
WB = [0, 32768, 65536, 98304, RT]   # src window bounds (rows)
NW = 4                # number of src windows
# one all-zero row inside each window (core-junk rows), window-local:
JROW_WIN = [12500, 2 * R + 12500 - WB[1], 5 * R + 12500 - WB[2],
            7 * R + 12500 - WB[3]]
SLOPE = 0.2
DM = 256              # feature dim of m_sim / d_sim
GSLOT = 49            # accumulator free-dim groups per parity buffer
WBMAX = 48            # max zsrc columns per compute batch
BTMAX = 8             # max tiles per compute batch
SGMAX = 32            # tiles per scatter group (4096-idx scatter calls)
GCALL = 1024          # max gather indices per call (hard ucode limit)

_nc_cache = {}


def _wrap16(idx16):
    """[n] int16 -> [128, n/16]: token i at [i%16, i//16], replicated to the
    8 Q7-core partition groups."""
    n = idx16.shape[0]
    w = np.ascontiguousarray(idx16.reshape(n // 16, 16).T)
    return np.tile(w, (8, 1))


# --------------------------------------------------------------------------
# host-side index preparation
# --------------------------------------------------------------------------

def _prep(src, dst):
    """Build the shared tile structure and per-core index streams.

    Returns (wseq, src16, q16, padf):
      wseq: per window, list of tile widths (shared across cores)
      src16[c]: [128, T/16] int16 wrapped gather idx per token
      q16[c]:   [128, Qp/16] int16 wrapped node idx per sub-row
      padf[c]:  [128, Qp/128] float32 pad count per sub-row
    """
    src = np.asarray(src).astype(np.int64)
    dst = np.asarray(dst).astype(np.int64)
    core = dst // NPC
    dloc = dst - core * NPC
    srcpad = (src // NPC) * R + (src % NPC)
    win = np.searchsorted(np.asarray(WB[1:4]), srcpad, side="right")
    sloc = (srcpad - np.asarray(WB)[win]).astype(np.int16)

    key = (core * NW + win) * NPC + dloc
    order = np.argsort(key, kind="stable")
    ks = key[order]
    sloc_s = sloc[order]
    # rank of each edge within its (core, win, node) run
    first = np.r_[True, ks[1:] != ks[:-1]]
    grp_start = np.maximum.accumulate(np.where(first, np.arange(E), 0))
    rank = np.arange(E) - grp_start

    uniq, counts = np.unique(ks, return_counts=True)
    c_u = uniq // (NW * NPC)
    k_u = (uniq // NPC) % NW
    n_u = uniq % NPC

    # per (core, win): sorted sub-row lengths
    lens = {}
    nodes = {}
    for c in range(C):
        for k in range(NW):
            m = (c_u == c) & (k_u == k)
            cnt = counts[m]
            nds = n_u[m]
            o = np.argsort(-cnt, kind="stable")
            lens[(c, k)] = cnt[o]
            nodes[(c, k)] = nds[o]

    # global tile structure
    wseq = []
    for k in range(NW):
        ntile = max(-(-len(lens[(c, k)]) // 128) for c in range(C))
        ws = []
        for g in range(ntile):
            w = 1
            for c in range(C):
                L = lens[(c, k)]
                if g * 128 < len(L):
                    w = max(w, int(L[g * 128]))
            ws.append(w)
        wseq.append(ws)

    tokbase = [0]
    qbase = [0]
    for k in range(NW):
        tokbase.append(tokbase[-1] + 128 * sum(wseq[k]))
        qbase.append(qbase[-1] + 128 * len(wseq[k]))
    T, Qp = tokbase[-1], qbase[-1]

    src16 = []
    q16 = []
    padf = []
    for c in range(C):
        s16 = np.empty(T, dtype=np.int16)
        for k in range(NW):
            s16[tokbase[k]:tokbase[k + 1]] = JROW_WIN[k]
        qq = np.empty(Qp, dtype=np.int16)
        qar = np.arange(Qp)
        qq[:] = (12500 + qar % 44).astype(np.int16)
        pf = np.zeros(Qp, dtype=np.float32)
        for k in range(NW):
            L = lens[(c, k)]
            Nd = nodes[(c, k)]
            nsub = len(L)
            ws = np.asarray(wseq[k], dtype=np.int64)
            cumw = np.concatenate([[0], np.cumsum(ws)])
            # per sub-row q: tile g = q//128, partition p = q%128
            q_ids = np.arange(nsub)
            g_ids = q_ids // 128
            qq[qbase[k] + q_ids] = Nd.astype(np.int16)
            pf[qbase[k] + q_ids] = (ws[g_ids] - L).astype(np.float32)
            pf[qbase[k] + nsub:qbase[k + 1]] = 0.0  # junk rows: denom junk anyway
            # per edge: token = tokbase[k] + (cumw[g] + rank)*128 + p
            lo = np.searchsorted(ks, (c * NW + k) * NPC)
            hi = np.searchsorted(ks, (c * NW + k + 1) * NPC)
            if hi > lo:
                nd_e = ks[lo:hi] % NPC            # node per edge
                r_e = rank[lo:hi]
                inv = np.empty(NPC, dtype=np.int64)
                inv[Nd] = q_ids
                q_e = inv[nd_e]
                tok = (tokbase[k]
                       + (cumw[q_e // 128] + r_e) * 128 + (q_e % 128))
                s16[tok] = sloc_s[lo:hi]
        src16.append(_wrap16(s16))
        q16.append(_wrap16(qq))
        padf.append(np.ascontiguousarray(
            pf.reshape(Qp // 128, 128).T).astype(np.float32))
    return wseq, src16, q16, padf


def _batches(wseq_k):
    """Split a window's tile-width run-lengths into (w, B) compute batches."""
    out = []
    i = 0
    while i < len(wseq_k):
        w = wseq_k[i]
        run = 1
        while i + run < len(wseq_k) and wseq_k[i + run] == w:
            run += 1
        maxb = max(1, min(BTMAX, WBMAX // w))
        for b0 in range(0, run, maxb):
            out.append((w, min(maxb, run - b0)))
        i += run
    return out


# --------------------------------------------------------------------------
# launch 1: projection  z_c [R, F] = xT_c.T @ wT_c  (pipelined x loads)
# --------------------------------------------------------------------------

def _build_proj_nc():
    nc = bacc.Bacc("TRN2", target_bir_lowering=False, debug=False,
                   num_devices=C)
    xT = nc.dram_tensor("xT", [DM, R], mybir.dt.float32,
                        kind="ExternalInput").ap()
    wT = nc.dram_tensor("wT", [DM, F], mybir.dt.float32,
                        kind="ExternalInput").ap()
    z_out = nc.dram_tensor("z", [R, F], mybir.dt.float32,
                           kind="ExternalOutput").ap()

    with tile.TileContext(nc) as tc:
        with ExitStack() as ctx:
            wp = ctx.enter_context(tc.tile_pool(name="w", bufs=1))
            xp = ctx.enter_context(tc.tile_pool(name="x", bufs=3))
            pp = ctx.enter_context(tc.tile_pool(name="ps", bufs=8,
                                                space="PSUM"))
            op = ctx.enter_context(tc.tile_pool(name="o", bufs=2))

            wt = wp.tile([128, 2, F], mybir.dt.float32)
            for j in range(2):
                nc.sync.dma_start(wt[:, j, :], wT[j * 128:(j + 1) * 128, :])

            ntiles = R // 128
            SB = 8
            z_r = z_out.rearrange("(t p) f -> p t f", p=128)
            for r0 in range(0, ntiles, SB):
                sb = min(SB, ntiles - r0)
                xt = xp.tile([128, 2, SB * 128], mybir.dt.float32, tag="xt")
                for j in range(2):
                    nc.sync.dma_start(
                        xt[:, j, :sb * 128],
                        xT[j * 128:(j + 1) * 128,
                           r0 * 128:(r0 + sb) * 128])
                ot = op.tile([128, SB, F], mybir.dt.float32, tag="ot")
                for t in range(sb):
                    ps = pp.tile([128, F], mybir.dt.float32, tag="ps")
                    for j in range(2):
                        nc.tensor.matmul(
                            out=ps[:],
                            lhsT=xt[:, j, t * 128:(t + 1) * 128],
                            rhs=wt[:, j, :],
                            start=(j == 0), stop=(j == 1))
                    nc.vector.tensor_copy(ot[:, t, :], ps[:])
                nc.sync.dma_start(z_r[:, r0:r0 + sb, :], ot[:, :sb, :])
    nc.compile()
    return nc


# --------------------------------------------------------------------------
# launch 2: edge phase
# --------------------------------------------------------------------------

def _build_edge_nc(wseq):
    T = 128 * sum(sum(ws) for ws in wseq)
    Qp = 128 * sum(len(ws) for ws in wseq)
    QTMAX = max(len(ws) for ws in wseq)

    nc = bacc.Bacc("TRN2", target_bir_lowering=False, debug=False,
                   num_devices=C, num_swdge_queues=4)
    z8 = nc.dram_tensor("z8", [RT, F], mybir.dt.float32,
                        kind="ExternalInput").ap()
    zown = nc.dram_tensor("zown", [R, F], mybir.dt.float32,
                          kind="ExternalInput").ap()
    s16d = nc.dram_tensor("s16", [128, T // 16], mybir.dt.int16,
                          kind="ExternalInput").ap()
    q16d = nc.dram_tensor("q16", [128, Qp // 16], mybir.dt.int16,
                          kind="ExternalInput").ap()
    padd = nc.dram_tensor("pad", [128, Qp // 128], mybir.dt.float32,
                          kind="ExternalInput").ap()
    h2 = nc.dram_tensor("h2", [2, 128, GSLOT, F], mybir.dt.float32,
                        kind="ExternalOutput").ap()

    qbase = [0]
    for k in range(NW):
        qbase.append(qbase[-1] + len(wseq[k]))

    with tile.TileContext(nc) as tc:
        with ExitStack() as ctx:
            ip = ctx.enter_context(tc.tile_pool(name="idx", bufs=1))
            ap_ = ctx.enter_context(tc.tile_pool(name="acc", bufs=1))
            dp = ctx.enter_context(tc.tile_pool(name="d", bufs=1))
            gp = ctx.enter_context(tc.tile_pool(name="g", bufs=3))
            ep = ctx.enter_context(tc.tile_pool(name="e", bufs=3))
            stp = ctx.enter_context(tc.tile_pool(name="st", bufs=2))

            # idx preload, per-window chunks so window 0 starts early
            s16t = ip.tile([128, T // 16], mybir.dt.int16, name="s16t")
            q16t = ip.tile([128, Qp // 16], mybir.dt.int16, name="q16t")
            padt = ip.tile([128, Qp // 128], mybir.dt.float32, name="padt")
            tb = 0
            qb = 0
            for k in range(NW):
                tk = 128 * sum(wseq[k])
                qk = 128 * len(wseq[k])
                nc.sync.dma_start(s16t[:, tb // 16:(tb + tk) // 16],
                                  s16d[:, tb // 16:(tb + tk) // 16])
                nc.sync.dma_start(q16t[:, qb // 16:(qb + qk) // 16],
                                  q16d[:, qb // 16:(qb + qk) // 16])
                nc.sync.dma_start(padt[:, qb // 128:(qb + qk) // 128],
                                  padd[:, qb // 128:(qb + qk) // 128])
                tb += tk
                qb += qk

            # accumulators: [rep][parity], rep alternates per scatter group
            acc = [[ap_.tile([128, GSLOT, F + 1], mybir.dt.bfloat16,
                             name=f"acc{r}{p}") for p in range(2)]
                   for r in range(2)]
            for r in range(2):
                for p in range(2):
                    nc.gpsimd.memset(acc[r][p][:], 0.0)

            tok_col = 0
            sgrp = 0
            grp_q0 = 0
            grp_j = 0
            grp_win = -1
            st = None

            def flush_scatter():
                nonlocal sgrp, grp_j, grp_q0, st
                if sgrp == 0:
                    return
                nc.gpsimd.dma_scatter_add(
                    acc[grp_j % 2][0][:], st[:, :sgrp, :],
                    q16t[:, grp_q0 * 8:(grp_q0 + sgrp) * 8],
                    sgrp * 128, sgrp * 128, F + 1,
                    queue_num=2, sbuf_tokens_per_rank=128,
                    parity_reg=0, out_ap_other=acc[grp_j % 2][1][:])
                grp_j += 1
                sgrp = 0
                st = None

            def stage_b(pb):
                """Post-exp work for a previous batch: weighted payload,
                tree-sum, denominator, scatter staging."""
                nonlocal sgrp, grp_q0, grp_win, st
                k, w, B, wsum, q_off, zb, exb = pb
                if st is not None and (sgrp + B > SGMAX or grp_win != k):
                    flush_scatter()
                if st is None:
                    st = stp.tile([128, SGMAX, F + 1],
                                  mybir.dt.bfloat16, tag="st")
                    grp_q0 = qbase[k] + q_off
                    grp_win = k
                with nc.allow_low_precision(reason="bf16 payload"):
                    nc.vector.tensor_mul(
                        zb[:, :wsum, :], zb[:, :wsum, :],
                        exb[:, :wsum].to_broadcast([128, wsum, F]))
                zb4 = zb[:, :wsum, :].rearrange("p (b w) f -> p b w f", b=B)
                cur_w = w
                with nc.allow_low_precision(reason="bf16 tree"):
                    while cur_w > 1:
                        hh = cur_w // 2
                        nc.vector.tensor_add(
                            zb4[:, :, 0:hh, :],
                            zb4[:, :, 0:hh, :],
                            zb4[:, :, cur_w - hh:cur_w, :])
                        cur_w -= hh
                nc.scalar.copy(st[:, sgrp:sgrp + B, 0:F], zb4[:, :, 0, :])
                # denominator in fp32, pad-correct, then round to bf16
                dn = ep.tile([128, BTMAX], mybir.dt.float32, tag="dn")
                nc.vector.tensor_reduce(
                    dn[:, :B],
                    exb[:, :wsum].rearrange("p (b w) -> p b w", b=B),
                    axis=mybir.AxisListType.X,
                    op=mybir.AluOpType.add)
                nc.vector.tensor_sub(
                    dn[:, :B], dn[:, :B],
                    padt[:, qbase[k] + q_off:qbase[k] + q_off + B])
                with nc.allow_low_precision(reason="bf16 partials"):
                    nc.vector.tensor_copy(st[:, sgrp:sgrp + B, F],
                                          dn[:, :B])
                sgrp += B
                if sgrp == SGMAX:
                    flush_scatter()

            gci = 0       # src gather call counter (queue 0/3 alternation)
            pending = None
            for k in range(NW):
                qt_k = len(wseq[k])
                nchunk = -(-qt_k // (GCALL // 128))
                # zdst preload chunks interleave with batches (queue 1)
                wt = dp.tile([128, QTMAX, F], mybir.dt.float32, tag="wt")
                issued = 0

                def issue_chunks(upto, wt=wt, qt_k=qt_k, k=k):
                    nonlocal issued
                    while issued < upto:
                        c0 = issued * (GCALL // 128)
                        cb = min(GCALL // 128, qt_k - c0)
                        nc.gpsimd.dma_gather(
                            wt[:, c0:c0 + cb, :], zown[:, :],
                            q16t[:, (qbase[k] + c0) * 8:
                                 (qbase[k] + c0 + cb) * 8],
                            cb * 128, cb * 128, F, queue_num=1)
                        issued += 1

                q_off = 0   # tile offset within this window
                for w, B in _batches(wseq[k]):
                    wsum = B * w
                    need = -(-(q_off + B) // (GCALL // 128))
                    issue_chunks(min(nchunk, need + 2))
                    zsrc = gp.tile([128, WBMAX, F], mybir.dt.float32,
                                   tag="zsrc")
                    for c0 in range(0, wsum, GCALL // 128):
                        nn = min(GCALL // 128, wsum - c0) * 128
                        i0 = (tok_col + c0) * 8
                        nc.gpsimd.dma_gather(
                            zsrc[:, c0:c0 + nn // 128, :],
                            z8[WB[k]:WB[k + 1], :],
                            s16t[:, i0:i0 + nn // 16], nn, nn, F,
                            queue_num=(0, 3)[gci % 2])
                        gci += 1

                    # bf16 payload copy early on Act (only needs zsrc)
                    zb = ep.tile([128, WBMAX, F], mybir.dt.bfloat16,
                                 tag="zb")
                    nc.scalar.copy(zb[:, :wsum, :], zsrc[:, :wsum, :])

                    prod = ep.tile([128, WBMAX, F], mybir.dt.float32,
                                   tag="prod")
                    zsrc4 = zsrc[:, :wsum, :].rearrange(
                        "p (b w) f -> p b w f", b=B)
                    prod4 = prod[:, :wsum, :].rearrange(
                        "p (b w) f -> p b w f", b=B)
                    zdst4 = wt[:, q_off:q_off + B, :].rearrange(
                        "p b (o f) -> p b o f", o=1)
                    nc.vector.tensor_mul(
                        prod4, zsrc4, zdst4.to_broadcast([128, B, w, F]))
                    ex = ep.tile([128, WBMAX], mybir.dt.float32, tag="ex")
                    nc.vector.tensor_reduce(
                        ex[:, :wsum], prod[:, :wsum, :],
                        axis=mybir.AxisListType.X, op=mybir.AluOpType.add)
                    nc.vector.scalar_tensor_tensor(
                        out=ex[:, :wsum], in0=ex[:, :wsum], scalar=SLOPE,
                        in1=ex[:, :wsum],
                        op0=mybir.AluOpType.mult, op1=mybir.AluOpType.max)
                    nc.vector.tensor_scalar_min(ex[:, :wsum], ex[:, :wsum],
                                                80.0)
                    nc.scalar.activation(ex[:, :wsum], ex[:, :wsum],
                                         mybir.ActivationFunctionType.Exp)
                    exb = ep.tile([128, WBMAX], mybir.dt.bfloat16, tag="exb")
                    nc.scalar.copy(exb[:, :wsum], ex[:, :wsum])

                    # software pipeline: post-exp work of the previous batch
                    if pending is not None:
                        stage_b(pending)
                    pending = (k, w, B, wsum, q_off, zb, exb)
                    q_off += B
                    tok_col += wsum
            if pending is not None:
                stage_b(pending)
            flush_scatter()

            # ---- normalize + elu + store (reuse compute pools) ----
            for p in range(2):
                a = acc[0][p]
                with nc.allow_low_precision(reason="bf16 acc merge"):
                    nc.vector.tensor_add(a[:], a[:], acc[1][p][:])
                for g0 in range(0, GSLOT, 25):
                    gl = min(25, GSLOT - g0)
                    rec = ep.tile([128, WBMAX], mybir.dt.float32, tag="ex")
                    nc.vector.tensor_copy(rec[:, :gl], a[:, g0:g0 + gl, F])
                    nc.vector.tensor_scalar_max(rec[:, :gl], rec[:, :gl],
                                                1e-30)
                    nc.vector.reciprocal(rec[:, :gl], rec[:, :gl])
                    hb = ep.tile([128, WBMAX, F], mybir.dt.float32,
                                 tag="prod")
                    h = hb[:, :gl, :]
                    nc.vector.tensor_copy(h, a[:, g0:g0 + gl, 0:F])
                    nc.vector.tensor_mul(
                        h, h, rec[:, :gl].to_broadcast([128, gl, F]))
                    nb = gp.tile([128, WBMAX, F], mybir.dt.float32,
                                 tag="zsrc")
                    hneg = nb[:, :gl, :]
                    nc.vector.tensor_scalar_min(hneg, h, 0.0)
                    nc.scalar.activation(hneg, hneg,
                                         mybir.ActivationFunctionType.Exp)
                    nc.vector.tensor_scalar_max(h, h, 0.0)
                    nc.vector.tensor_add(h, h, hneg)
                    nc.vector.tensor_scalar_add(h, h, -1.0)
                    nc.sync.dma_start(h2[p, :, g0:g0 + gl, :], h)
    nc.compile()
    return nc


# --------------------------------------------------------------------------
# entry point
# --------------------------------------------------------------------------

def kernel(m_sim, d_sim, Wm, Wd, src, dst, _profile=None):
    m_sim = np.asarray(m_sim, dtype=np.float32)
    d_sim = np.asarray(d_sim, dtype=np.float32)
    Wm = np.asarray(Wm, dtype=np.float32)
    Wd = np.asarray(Wd, dtype=np.float32)

    wseq, src16, q16, padf = _prep(src, dst)

    # ---- launch 1: projection ----
    if "proj" not in _nc_cache:
        _nc_cache["proj"] = _build_proj_nc()
    proj_nc = _nc_cache["proj"]

    x = np.concatenate([m_sim, d_sim], axis=0)        # [N, DM]
    wmT = np.ascontiguousarray(Wm.T)                  # [DM, F]
    wdT = np.ascontiguousarray(Wd.T)
    in1 = []
    for c in range(C):
        xT_c = np.zeros((DM, R), dtype=np.float32)
        xT_c[:, :NPC] = x[c * NPC:(c + 1) * NPC].T
        in1.append({"xT": xT_c, "wT": wmT if c < 4 else wdT})
    r1 = bass_utils.run_bass_kernel_spmd(proj_nc, in1,
                                         core_ids=list(range(C)),
                                         **(_profile or {}))
    z8_full = np.concatenate([r1.results[c]["z"] for c in range(C)],
                             axis=0)                  # [RT, F]

    # ---- launch 2: edge phase ----
    key = ("edge", tuple(tuple(ws) for ws in wseq))
    if key not in _nc_cache:
        _nc_cache[key] = _build_edge_nc(wseq)
    edge_nc = _nc_cache[key]

    in2 = []
    for c in range(C):
        in2.append({
            "z8": z8_full,
            "zown": z8_full[c * R:(c + 1) * R],
            "s16": src16[c],
            "q16": q16[c],
            "pad": padf[c],
        })
    r2 = bass_utils.run_bass_kernel_spmd(edge_nc, in2,
                                         core_ids=list(range(C)),
                                         **(_profile or {}))
    h = np.empty((N, F), dtype=np.float32)
    for c in range(C):
        h2 = r2.results[c]["h2"]                      # [2, 128, GSLOT, 64]
        # node n (local) at [ (n>>7)&1, n&127, n>>8 ]
        n_ = np.arange(NPC)
        h[c * NPC:(c + 1) * NPC] = h2[(n_ >> 7) & 1, n_ & 127, n_ >> 8]
    kernel._last_results = (r1, r2)
    return h
